# revision 1
# baseline (speedup 1.0000x reference)
"""Trainium2 Bass kernel for a 6-layer GPT (D=512, H=8, T=1024, B=2, V=50257).

Strategy (8 NeuronCores):
- Token-shard the transformer body: core c owns 256 tokens (cores 0-3 =
  batch 0 chunks 0-3, cores 4-7 = batch 1 chunks 0-3).
- Per layer: LN1 -> QKV (q/k in transposed [dim, tok] layout, v natural)
  -> AllGather K,V across the 4-core batch group -> causal attention in
  transposed layout (softmax denominator fused as a ones-row of V; no
  on-chip transposes in the attention loop) -> Wo + residual -> LN2 ->
  MLP (mm1 transposed so mm2 needs no transpose) -> residual.
- Final LN -> AllGather hidden state across all 8 cores -> vocab-sharded
  LM head: core c computes logits[:, c*6656:(c+1)*6656] (Wlm zero-padded
  to 8*6656 columns host-side).
- Host folds LN gamma/beta and the 1/sqrt(HS) score scale into the
  weights; embedding gather happens host-side (tiny).
"""

import numpy as np

import concourse.bass as bass
import concourse.tile as tile
from concourse import bacc, mybir
from concourse import bass_utils
from concourse.bass import ds, ts
from concourse.masks import make_identity

FP = mybir.dt.float32
AF = mybir.ActivationFunctionType
OP = mybir.AluOpType

V, D, T, L, H, HS, B = 50257, 512, 1024, 6, 8, 64, 2
FF = 4 * D
EPS = 1e-5
NC = 8          # cores
CH = 256        # tokens per core
VS = 6284       # padded vocab shard per core; 8*VS = 50272 >= V
KD = D // 128   # 4 k-tiles over D
MD = FF // 128  # 16 m-tiles over FF
NEG = -1.0e9


def build_program(reps=1, with_bias=True, layers=L, with_head=True, with_attn=True, head_mode='full'):
    nc = bacc.Bacc("TRN2", target_bir_lowering=False, debug=False, num_devices=NC)

    # ---- I/O ----
    x0 = nc.dram_tensor("x0", [CH, D], FP, kind="ExternalInput").ap()
    wq = nc.dram_tensor("wq", [L, D, D], FP, kind="ExternalInput").ap()
    wk = nc.dram_tensor("wk", [L, D, D], FP, kind="ExternalInput").ap()
    wv = nc.dram_tensor("wv", [L, D, D], FP, kind="ExternalInput").ap()
    wo = nc.dram_tensor("wo", [L, D, D], FP, kind="ExternalInput").ap()
    w1 = nc.dram_tensor("w1", [L, D, FF], FP, kind="ExternalInput").ap()
    w2 = nc.dram_tensor("w2", [L, FF, D], FP, kind="ExternalInput").ap()
    wlm = nc.dram_tensor("wlm", [D, VS], FP, kind="ExternalInput").ap()
    # bqk[p, l, 0/1, kd]: per-partition bias for qT/kT ([D] rearranged)
    bqk = nc.dram_tensor("bqk", [128, L, 2, KD], FP, kind="ExternalInput").ap()
    b1t = nc.dram_tensor("b1t", [128, L, MD], FP, kind="ExternalInput").ap()
    # bo2[l, 0]=bo_eff, [l, 1]=b2 (free-dim biases, broadcast via DMA)
    bo2 = nc.dram_tensor("bo2", [L, 2, D], FP, kind="ExternalInput").ap()
    # causal mask per core: [p, kchunk, ktile, 256 queries]
    msk = nc.dram_tensor("msk", [128, 4, 2, CH], FP, kind="ExternalInput").ap()
    logits = nc.dram_tensor("logits", [B * T, VS], FP, kind="ExternalOutput").ap()

    KV_K = 128 * KD * CH            # kT flat size per core
    KV_V = 128 * 2 * H * 65         # v_aug flat size per core
    KV = KV_K + KV_V
    XF = 128 * KD * CH              # xfT flat size

    from contextlib import ExitStack
    with ExitStack() as stk:
        tc = stk.enter_context(tile.TileContext(nc))
        ec = stk.enter_context
        consts = ec(tc.tile_pool(name="consts", bufs=1))
        xpool = ec(tc.tile_pool(name="xpool", bufs=1))
        hpool = ec(tc.tile_pool(name="hpool", bufs=2))
        t4 = ec(tc.tile_pool(name="t4", bufs=4))          # [128,KD,CH] transposed acts
        w4pool = ec(tc.tile_pool(name="w4", bufs=4))      # [128,KD,512] weights
        w1pool = ec(tc.tile_pool(name="w1k", bufs=4))     # [128,FF] W1 k-tiles
        w2pool = ec(tc.tile_pool(name="w2k", bufs=4))     # [128,512] W2 k-tiles
        kvall = ec(tc.tile_pool(name="kvall", bufs=1))
        vaugp = ec(tc.tile_pool(name="vaug", bufs=1))
        small = ec(tc.tile_pool(name="small", bufs=2))
        expp = ec(tc.tile_pool(name="exp", bufs=3))
        gtp = ec(tc.tile_pool(name="gt", bufs=2))
        lgp = ec(tc.tile_pool(name="lg", bufs=2))
        bcp = ec(tc.tile_pool(name="bcast", bufs=2))
        xftp = ec(tc.tile_pool(name="xft", bufs=2))
        mmp = ec(tc.tile_pool(name="mm", bufs=2, space="PSUM"))
        avp = ec(tc.tile_pool(name="avp", bufs=2, space="PSUM"))
        spp = ec(tc.tile_pool(name="sp", bufs=2, space="PSUM"))
        mop = ec(tc.tile_pool(name="mo", bufs=2, space="PSUM"))
        avacc = ec(tc.tile_pool(name="avacc", bufs=1))
        dram = ec(tc.tile_pool(name="dram", bufs=2, space="DRAM"))
        if True:
            ident = consts.tile([128, 128], FP)
            make_identity(nc, ident)
            epst = consts.tile([128, 1], FP)
            nc.vector.memset(epst, EPS)
            bqk_sb = consts.tile([128, L, 2, KD], FP)
            nc.sync.dma_start(bqk_sb[:], bqk)
            b1_sb = consts.tile([128, L, MD], FP)
            nc.sync.dma_start(b1_sb[:], b1t)
            msk_sb = consts.tile([128, 4, 2, CH], FP)
            nc.sync.dma_start(msk_sb[:], msk)



            def layernorm(src, tag):
                """src [128,2,D] -> fresh normalized tile [128,2,D] (no affine)."""
                out = hpool.tile([128, 2, D], FP, tag="h")
                for tt in range(2):
                    st = small.tile([128, 6], FP, tag="bnst")
                    nc.vector.bn_stats(st[:], src[:, tt, :])
                    mv = small.tile([128, 2], FP, tag="bnmv")
                    nc.vector.bn_aggr(mv[:], st[:])
                    nc.scalar.activation(mv[:, 1:2], mv[:, 1:2], AF.Sqrt,
                                         bias=epst[:, 0:1])
                    nc.vector.reciprocal(mv[:, 1:2], mv[:, 1:2])
                    nc.vector.tensor_scalar(
                        out=out[:, tt, :], in0=src[:, tt, :],
                        scalar1=mv[:, 0:1], scalar2=mv[:, 1:2],
                        op0=OP.subtract, op1=OP.mult)
                return out

            def transpose2(src, tag):
                """src [128,2,D] (tokens, dims) -> [128,KD,CH] (dims, tokens)."""
                out = t4.tile([128, KD, CH], FP, tag="t4")
                for d in range(KD):
                    for tt in range(2):
                        ps = mmp.tile([128, 512], FP, tag="mm")
                        nc.tensor.transpose(ps[:, :128], src[:, tt, ds(d * 128, 128)],
                                            ident[:])
                        nc.vector.tensor_copy(out[:, d, ds(tt * 128, 128)],
                                              ps[:, :128])
                return out

            def load_w4(src_ap, tag="w4"):
                w = w4pool.tile([128, KD, 512], FP, tag=tag)
                nc.sync.dma_start(w[:], src_ap.rearrange("(ko p) m -> p ko m", p=128))
                return w

            for rep in range(reps):
              xt = xpool.tile([128, 2, D], FP, tag="xt", name=f"xt{rep}")
              nc.sync.dma_start(xt[:], x0.rearrange("(tt p) d -> p tt d", p=128))
              for l in range(layers):
                # ---- LN1 + transpose ----
                h = layernorm(xt, "h")
                hT = transpose2(h, "t4")

                # ---- k/v first so the AllGather starts ASAP ----
                wk_sb = load_w4(wk[l])
                wv_sb = load_w4(wv[l])
                kT = t4.tile([128, KD, CH], FP, tag="t4")
                for d in range(KD):
                    ps = mmp.tile([128, 512], FP, tag="mm")
                    for k in range(KD):
                        nc.tensor.matmul(ps[:, :CH], wk_sb[:, k, ds(d * 128, 128)],
                                         hT[:, k, :], start=(k == 0),
                                         stop=(k == KD - 1))
                    nc.vector.tensor_scalar_add(kT[:, d, :], ps[:, :CH],
                                                bqk_sb[:, l, 1, d:d + 1])
                vaug = vaugp.tile([128, 2, H, 65], FP)
                nc.vector.memset(vaug[:, :, :, 64:65], 1.0)
                for tt in range(2):
                    ps = mmp.tile([128, 512], FP, tag="mm")
                    for k in range(KD):
                        nc.tensor.matmul(ps[:, :D], hT[:, k, ds(tt * 128, 128)],
                                         wv_sb[:, k, :], start=(k == 0),
                                         stop=(k == KD - 1))
                    for hh in range(H):
                        nc.vector.tensor_copy(vaug[:, tt, hh, 0:64],
                                              ps[:, ds(hh * 64, 64)])

                # ---- AllGather K,V across batch group ----
                kv_in = dram.tile([KV], FP, tag="kvin")
                nc.sync.dma_start(
                    kv_in[0:KV_K].rearrange("(p a b) -> p a b", p=128, a=KD), kT[:])
                nc.sync.dma_start(
                    kv_in[KV_K:KV].rearrange("(p a h e) -> p a h e", p=128, a=2, h=H),
                    vaug[:])
                kv_out = dram.tile([4, KV], FP, tag="kvout")
                nc.gpsimd.collective_compute(
                    "AllGather", OP.bypass,
                    replica_groups=[[0, 1, 2, 3], [4, 5, 6, 7]],
                    ins=[kv_in[:].opt()], outs=[kv_out[:].opt()])
                # ---- q projection overlaps the collective ----
                wq_sb = load_w4(wq[l])
                qT = t4.tile([128, KD, CH], FP, tag="t4")
                for d in range(KD):
                    ps = mmp.tile([128, 512], FP, tag="mm")
                    for k in range(KD):
                        nc.tensor.matmul(ps[:, :CH], wq_sb[:, k, ds(d * 128, 128)],
                                         hT[:, k, :], start=(k == 0),
                                         stop=(k == KD - 1))
                    nc.vector.tensor_scalar_add(qT[:, d, :], ps[:, :CH],
                                                bqk_sb[:, l, 0, d:d + 1])

                if not with_attn:
                    continue
                kTall = kvall.tile([128, KD, 4, CH], FP, tag="ktall")
                vall = kvall.tile([128, 4, 2, H, 65], FP, tag="vall")
                for c in range(4):
                    nc.sync.dma_start(
                        kTall[:, :, c, :],
                        kv_out[c, 0:KV_K].rearrange("(p a b) -> p a b", p=128, a=KD))
                    nc.sync.dma_start(
                        vall[:, c, :, :, :],
                        kv_out[c, KV_K:KV].rearrange("(p a h e) -> p a h e",
                                                     p=128, a=2, h=H))

                # ---- attention (transposed layout) ----
                # av accumulation lives in SBUF (PSUM is only 8 banks);
                # per-chunk AV psum tiles are transient.
                av_acc = avacc.tile([65, H, CH], FP, tag="avacc")
                for c in range(4):
                    for hh in range(H):
                        pb = (hh % 2) * 64
                        dt_ = hh // 2
                        avps = avp.tile([65, CH], FP, tag="av")
                        for kt in range(2):
                            sps = spp.tile([128, CH], FP, tag="sp")
                            nc.tensor.matmul(
                                sps[:],
                                kTall[pb:pb + 64, dt_, c, ds(kt * 128, 128)],
                                qT[pb:pb + 64, dt_, :],
                                start=True, stop=True)
                            ex = expp.tile([128, CH], FP, tag="exp")
                            nc.scalar.activation(ex[:], sps[:], AF.Exp)
                            nc.gpsimd.tensor_tensor(ex[:], ex[:],
                                                    msk_sb[:, c, kt, :], OP.mult)
                            nc.tensor.matmul(
                                avps[:], vall[:, c, kt, hh, :], ex[:],
                                start=(kt == 0), stop=(kt == 1))
                        if c == 0:
                            nc.vector.tensor_copy(av_acc[:, hh, :], avps[:])
                        else:
                            nc.vector.tensor_tensor(av_acc[:, hh, :],
                                                    av_acc[:, hh, :], avps[:],
                                                    OP.add)

                # ---- normalize by denominators (all in base-0 partition space;
                # DVE cannot shift partitions, DMA can) ----
                rdram = dram.tile([8, CH], FP, tag="rdram")
                nc.sync.dma_start(rdram[:], av_acc[64:65, :, :])
                rbc0 = avacc.tile([64, H, CH], FP, tag="rbc0")
                for hh in range(H):
                    nc.sync.dma_start(
                        rbc0[:, hh, :],
                        bass.AP(tensor=rdram.tensor, offset=rdram.offset + hh * CH,
                                ap=[[0, 64], [1, CH]]))
                nc.vector.reciprocal(rbc0[:], rbc0[:])
                attn_n = rbc0
                nc.vector.tensor_tensor(attn_n[:], av_acc[0:64, :, :], rbc0[:],
                                        OP.mult)
                attT = t4.tile([128, KD, CH], FP, tag="t4")
                for dt_ in range(KD):
                    for a in range(2):
                        nc.sync.dma_start(attT[a * 64:(a + 1) * 64, dt_, :],
                                          attn_n[:, 2 * dt_ + a, :])

                # ---- Wo + bias + residual ----
                wo_sb = load_w4(wo[l])
                if with_bias:
                    bo_b = bcp.tile([128, D], FP, tag="bc")
                    bo_src = bo2[l, 0]
                    nc.sync.dma_start(bo_b[:], bass.AP(
                        tensor=bo_src.tensor, offset=bo_src.offset,
                        ap=[[0, 128]] + list(bo_src.ap)))
                for tt in range(2):
                    ps = mmp.tile([128, 512], FP, tag="mm")
                    for k in range(KD):
                        nc.tensor.matmul(ps[:, :D], attT[:, k, ds(tt * 128, 128)],
                                         wo_sb[:, k, :], start=(k == 0),
                                         stop=(k == KD - 1))
                    if with_bias:
                        nc.vector.tensor_tensor(ps[:, :D], ps[:, :D], bo_b[:],
                                                OP.add)
                    nc.vector.tensor_tensor(xt[:, tt, :], xt[:, tt, :], ps[:, :D],
                                            OP.add)

                # ---- LN2 + transpose ----
                h2 = layernorm(xt, "h")
                h2T = transpose2(h2, "t4")

                # ---- MLP ----
                w1_sb = [w1pool.tile([128, FF], FP, tag="w1k", name=f"w1k{l}_{kk}") for kk in range(KD)]
                for k in range(KD):
                    nc.sync.dma_start(
                        w1_sb[k][:],
                        w1[l].rearrange("(ko p) f -> p ko f", p=128)[:, k, :])
                if with_bias:
                    b2_b = bcp.tile([128, D], FP, tag="bc")
                    b2_src = bo2[l, 1]
                    nc.sync.dma_start(b2_b[:], bass.AP(
                        tensor=b2_src.tensor, offset=b2_src.offset,
                        ap=[[0, 128]] + list(b2_src.ap)))
                x2ps = [mop.tile([128, D], FP, tag="mo", name=f"mo{l}_{kk}") for kk in range(2)]
                for m in range(MD):
                    gps = mmp.tile([128, 512], FP, tag="mm")
                    for k in range(KD):
                        nc.tensor.matmul(gps[:, :CH], w1_sb[k][:, ds(m * 128, 128)],
                                         h2T[:, k, :], start=(k == 0),
                                         stop=(k == KD - 1))
                    gt = gtp.tile([128, CH], FP, tag="gt")
                    nc.scalar.activation(gt[:], gps[:, :CH], AF.Gelu,
                                         bias=b1_sb[:, l, m:m + 1])
                    w2t = w2pool.tile([128, D], FP, tag="w2k")
                    nc.sync.dma_start(
                        w2t[:], w2[l].rearrange("(ko p) d -> p ko d", p=128)[:, m, :])
                    for tt in range(2):
                        nc.tensor.matmul(x2ps[tt][:], gt[:, ds(tt * 128, 128)],
                                         w2t[:], start=(m == 0), stop=(m == MD - 1))
                for tt in range(2):
                    if with_bias:
                        nc.vector.tensor_tensor(x2ps[tt][:], x2ps[tt][:], b2_b[:],
                                                OP.add)
                    nc.vector.tensor_tensor(xt[:, tt, :], xt[:, tt, :], x2ps[tt][:],
                                            OP.add)

              if not with_head:
                  continue
              # ---- final LN + AllGather hidden ----
              xf = layernorm(xt, "h")
              xfT = transpose2(xf, "t4")
              xf_in = dram.tile([XF], FP, tag="xfin")
              nc.sync.dma_start(
                  xf_in[:].rearrange("(p a b) -> p a b", p=128, a=KD), xfT[:])
              xf_out = dram.tile([NC, XF], FP, tag="xfout", addr_space="Shared")
              nc.gpsimd.collective_compute(
                  "AllGather", OP.bypass,
                  replica_groups=[list(range(NC))],
                  ins=[xf_in[:].opt()], outs=[xf_out[:].opt()])
              xfall = []
              for cg in range(2):
                  xa = xftp.tile([128, KD, 4, CH], FP, tag="xft")
                  for c in range(4):
                      nc.sync.dma_start(
                          xa[:, :, c, :],
                          xf_out[cg * 4 + c, :].rearrange("(p a b) -> p a b",
                                                          p=128, a=KD))
                  xfall.append(xa)

              # ---- LM head: vocab-sharded (ragged last tile) ----
              NT = (VS + 511) // 512
              hp_pools = [mmp, mop, spp]
              gi = 0
              for n in range(NT):
                  nsz = min(512, VS - n * 512)
                  wl = w4pool.tile([128, KD, 512], FP, tag="w4")
                  if head_mode != "nowlm" or n == 0:
                      nc.sync.dma_start(
                          wl[:, :, :nsz],
                          wlm.rearrange("(ko p) v -> p ko v", p=128)[:, :, ds(n * 512, nsz)])
                  for c in range(NC):
                      for mt in range(2):
                          pool_i, ptag = (
                              (mmp, "mm"), (mop, "mo"), (spp, "sp"))[gi % 3]
                          gi += 1
                          ps = pool_i.tile([128, 512], FP, tag=ptag,
                                           name=f"hps{n}_{c}_{mt}")
                          for k in range(KD):
                              nc.tensor.matmul(
                                  ps[:, :nsz],
                                  xfall[c // 4][:, k, c % 4, ds(mt * 128, 128)],
                                  wl[:, k, :nsz], start=(k == 0), stop=(k == KD - 1))
                          lg = w2pool.tile([128, 512], FP, tag="w2k",
                                           name=f"lg{n}_{c}_{mt}")
                          if head_mode != "nocopy":
                              nc.vector.tensor_copy(lg[:, :nsz], ps[:, :nsz])
                          if head_mode != "nodma":
                              nc.sync.dma_start(
                                  logits[ds(c * CH + mt * 128, 128),
                                         ds(n * 512, nsz)],
                                  lg[:, :nsz])
    nc.compile()
    return nc


_CACHE = {}


def _get_program(with_bias=True):
    key = ("nc", with_bias)
    if key not in _CACHE:
        _CACHE[key] = build_program(with_bias=with_bias)
    return _CACHE[key]


def _prep_inputs(inputs):
    f = lambda k: np.asarray(inputs[k], np.float32)
    idx = np.asarray(inputs["idx"]).astype(np.int64)
    tok_emb, pos_emb = f("tok_emb"), f("pos_emb")
    x0 = tok_emb[idx] + pos_emb[None, :T]          # [B, T, D]
    x0 = x0.reshape(NC, CH, D)

    ln1_g, ln1_b = f("ln1_g"), f("ln1_b")
    ln2_g, ln2_b = f("ln2_g"), f("ln2_b")
    Wq, bq = f("Wq"), f("bq")
    Wk, bk = f("Wk"), f("bk")
    Wv, bv = f("Wv"), f("bv")
    Wo, bo = f("Wo"), f("bo")
    W1, b1 = f("W1"), f("b1")
    W2, b2 = f("W2"), f("b2")
    lnf_g, lnf_b = f("lnf_g"), f("lnf_b")
    Wlm, blm = f("Wlm"), f("blm")

    sc = 1.0 / np.sqrt(HS)
    wqe = ln1_g[:, :, None] * Wq * sc
    bqe = (np.einsum("ld,ldm->lm", ln1_b, Wq) + bq) * sc
    wke = ln1_g[:, :, None] * Wk
    bke = np.einsum("ld,ldm->lm", ln1_b, Wk) + bk
    wve = ln1_g[:, :, None] * Wv
    bve = np.einsum("ld,ldm->lm", ln1_b, Wv) + bv
    boe = np.einsum("lm,lmd->ld", bve, Wo) + bo
    w1e = ln2_g[:, :, None] * W1
    b1e = np.einsum("ld,ldf->lf", ln2_b, W1) + b1
    wlme = lnf_g[:, None] * Wlm
    blme = lnf_b @ Wlm + blm

    bqk = np.stack([bqe, bke], axis=1)             # [L, 2, D]
    bqk = bqk.reshape(L, 2, KD, 128).transpose(3, 0, 1, 2).copy()
    b1t = b1e.reshape(L, MD, 128).transpose(2, 0, 1).copy()
    bo2 = np.stack([boe, b2], axis=1)              # [L, 2, D]

    wlmp = np.zeros((D, NC * VS), np.float32)
    wlmp[:, :V] = wlme

    # causal 0/1 masks per core (multiplied in after exp): [p, kc, kt, q]
    masks = []
    for core in range(NC):
        cc = core % 4
        qpos = cc * CH + np.arange(CH)
        m = np.empty((128, 4, 2, CH), np.float32)
        for kc in range(4):
            for kt in range(2):
                kpos = kc * CH + kt * 128 + np.arange(128)
                m[:, kc, kt, :] = (kpos[:, None] <= qpos[None, :]).astype(np.float32)
        masks.append(m)

    shared = dict(wq=np.ascontiguousarray(wqe), wk=np.ascontiguousarray(wke),
                  wv=np.ascontiguousarray(wve), wo=np.ascontiguousarray(Wo),
                  w1=np.ascontiguousarray(w1e), w2=np.ascontiguousarray(W2),
                  bqk=bqk, b1t=b1t, bo2=np.ascontiguousarray(bo2))
    in_maps = []
    for core in range(NC):
        m = dict(shared)
        m["x0"] = np.ascontiguousarray(x0[core])
        m["msk"] = masks[core]
        m["wlm"] = np.ascontiguousarray(wlmp[:, core * VS:(core + 1) * VS])
        in_maps.append(m)
    return in_maps, blme


def _run(inputs, trace=False):
    in_maps, blme = _prep_inputs(inputs)
    with_bias = bool(np.any(in_maps[0]["bo2"]))
    nc = _get_program(with_bias=with_bias)
    res = bass_utils.run_bass_kernel_spmd(nc, in_maps, core_ids=list(range(NC)),
                                          trace=trace)
    lg = np.concatenate([res.results[c]["logits"] for c in range(NC)], axis=1)
    out = lg[:, :V]
    if np.any(blme):
        out = out + blme[None, :]
    return out.reshape(B, T, V).astype(np.float32), res


def kernel(**inputs) -> np.ndarray:
    out, _ = _run(inputs, trace=False)
    return out



# revision 9
# speedup vs baseline: 2.3458x; 2.3458x over previous
"""Trainium2 Bass kernel for a 6-layer GPT (D=512, H=8, T=1024, B=2, V=50257).

Strategy (8 NeuronCores), v2 = bf16 matmul path everywhere:
- Token-shard the transformer body: core c owns 256 tokens (cores 0-3 =
  batch 0 chunks 0-3, cores 4-7 = batch 1 chunks 0-3).
- All matmul operands are bf16 (PSUM accumulates fp32); residual stream
  and LN statistics stay fp32.
- Per layer: LN1 -> QKV (q/k transposed [dim, tok], v natural) ->
  AllGather K,V (bf16) across the 4-core batch group -> causal attention
  with per-head PSUM accumulation (softmax denominator = ones-column of
  V; 0/1 masks multiplied after exp) -> Wo + residual -> LN2 -> MLP ->
  residual.  Next layer's weights are prefetched during attention.
- Final LN -> AllToAll-rotated exchange of the hidden state so every
  core sees the other 7 chunks with its own chunk logically last ->
  vocab-sharded LM head (Wlm resident in SBUF, loaded once at start):
  pass 1 computes the core's own 256 tokens (overlapping the AllToAll),
  pass 2 the remaining 7 chunks.  Logits are written bf16; the host
  reorders the rotated row blocks and upconverts.
- Host folds LN gamma/beta and the 1/sqrt(HS) score scale into the
  weights; embedding gather happens host-side (tiny).
"""

import numpy as np
import ml_dtypes

import concourse.bass as bass
import concourse.tile as tile
from concourse import bacc, mybir
from concourse import bass_utils
from concourse.bass import ds, ts
from concourse.masks import make_identity

FP = mybir.dt.float32
BF = mybir.dt.bfloat16
AF = mybir.ActivationFunctionType
OP = mybir.AluOpType

V, D, T, L, H, HS, B = 50257, 512, 1024, 6, 8, 64, 2
FF = 4 * D
EPS = 1e-5
NC = 8          # cores
CH = 256        # tokens per core
VS = 6284       # padded vocab shard per core; 8*VS = 50272 >= V
KD = D // 128   # 4 k-tiles over D
MD = FF // 128  # 16 m-tiles over FF
BT = B * T


def build_program(with_bias=True, layers=L):
    nc = bacc.Bacc("TRN2", target_bir_lowering=False, debug=False, num_devices=NC)

    # ---- I/O ----
    x0 = nc.dram_tensor("x0", [CH, D], FP, kind="ExternalInput").ap()
    wq = nc.dram_tensor("wq", [L, D, D], BF, kind="ExternalInput").ap()
    wk = nc.dram_tensor("wk", [L, D, D], BF, kind="ExternalInput").ap()
    wv = nc.dram_tensor("wv", [L, D, D], BF, kind="ExternalInput").ap()
    wo = nc.dram_tensor("wo", [L, D, D], BF, kind="ExternalInput").ap()
    w1 = nc.dram_tensor("w1", [L, D, FF], BF, kind="ExternalInput").ap()
    w2 = nc.dram_tensor("w2", [L, FF, D], BF, kind="ExternalInput").ap()
    wlm = nc.dram_tensor("wlm", [D, VS], BF, kind="ExternalInput").ap()
    # bqk[p, l, 0/1, kd]: per-partition bias for qT/kT ([D] rearranged)
    bqk = nc.dram_tensor("bqk", [128, L, 2, KD], FP, kind="ExternalInput").ap()
    b1t = nc.dram_tensor("b1t", [128, L, MD], FP, kind="ExternalInput").ap()
    # bo2[l, 0]=bo_eff, [l, 1]=b2 (free-dim biases, broadcast via DMA)
    bo2 = nc.dram_tensor("bo2", [L, 2, D], FP, kind="ExternalInput").ap()
    # causal mask per core: [p, kchunk, ktile, 256 queries] (0/1 in bf16)
    msk = nc.dram_tensor("msk", [128, 4, 2, CH], BF, kind="ExternalInput").ap()
    logits = nc.dram_tensor("logits", [BT, VS], BF, kind="ExternalOutput").ap()

    KV_K = 128 * KD * CH            # kT flat size per core (elements)
    KV_V = 128 * 2 * H * 65         # v_aug flat size per core
    KV = KV_K + KV_V
    XF = 128 * KD * CH              # xfT flat size

    from contextlib import ExitStack
    with ExitStack() as stk:
        tc = stk.enter_context(tile.TileContext(nc))
        ec = stk.enter_context
        consts = ec(tc.tile_pool(name="consts", bufs=1))
        wlmp = ec(tc.tile_pool(name="wlmp", bufs=1))
        xpool = ec(tc.tile_pool(name="xpool", bufs=1))
        hpool = ec(tc.tile_pool(name="hpool", bufs=2))
        t4 = ec(tc.tile_pool(name="t4", bufs=4))          # [128,KD,CH] transposed acts
        wqkvop = ec(tc.tile_pool(name="wqkvo", bufs=2))   # [128,KD,4,512] per-layer qkvo
        w1pool = ec(tc.tile_pool(name="w1k", bufs=2))     # [128,KD,FF]
        w2pool = ec(tc.tile_pool(name="w2k", bufs=4))     # [128,512] W2 m-tiles
        kvall = ec(tc.tile_pool(name="kvall", bufs=1))
        vaugp = ec(tc.tile_pool(name="vaug", bufs=1))
        small = ec(tc.tile_pool(name="small", bufs=2))
        expp = ec(tc.tile_pool(name="exp", bufs=3))
        gtp = ec(tc.tile_pool(name="gt", bufs=2))
        attp = ec(tc.tile_pool(name="attp", bufs=1))      # att_acc + dn + rbc
        lgp = ec(tc.tile_pool(name="lg", bufs=4))
        bcp = ec(tc.tile_pool(name="bcast", bufs=2))
        xftp = ec(tc.tile_pool(name="xft", bufs=1))
        xfap = ec(tc.tile_pool(name="xfa", bufs=2))
        mmp = ec(tc.tile_pool(name="mm", bufs=2, space="PSUM"))
        avp = ec(tc.tile_pool(name="avp", bufs=2, space="PSUM"))
        spp = ec(tc.tile_pool(name="sp", bufs=2, space="PSUM"))
        mop = ec(tc.tile_pool(name="mo", bufs=2, space="PSUM"))
        dram = ec(tc.tile_pool(name="dram", bufs=2, space="DRAM"))

        ident = consts.tile([128, 128], BF)
        make_identity(nc, ident)
        epst = consts.tile([128, 1], FP)
        nc.vector.memset(epst, EPS)
        bqk_sb = consts.tile([128, L, 2, KD], FP)
        nc.sync.dma_start(bqk_sb[:], bqk)
        b1_sb = consts.tile([128, L, MD], FP)
        nc.sync.dma_start(b1_sb[:], b1t)
        msk_sb = consts.tile([128, 4, 2, CH], BF)
        nc.sync.dma_start(msk_sb[:], msk)

        def layernorm(src, tag):
            """src [128,2,D] fp32 -> fresh normalized tile [128,2,D] bf16."""
            out = hpool.tile([128, 2, D], BF, tag="h")
            for tt in range(2):
                st = small.tile([128, 6], FP, tag="bnst")
                nc.vector.bn_stats(st[:], src[:, tt, :])
                mv = small.tile([128, 2], FP, tag="bnmv")
                nc.vector.bn_aggr(mv[:], st[:])
                nc.scalar.activation(mv[:, 1:2], mv[:, 1:2], AF.Sqrt,
                                     bias=epst[:, 0:1])
                nc.vector.reciprocal(mv[:, 1:2], mv[:, 1:2])
                nc.vector.tensor_scalar(
                    out=out[:, tt, :], in0=src[:, tt, :],
                    scalar1=mv[:, 0:1], scalar2=mv[:, 1:2],
                    op0=OP.subtract, op1=OP.mult)
            return out

        def transpose2(src, pool=t4):
            """src [128,2,D] bf16 (tokens, dims) -> [128,KD,CH] bf16 (dims, toks)."""
            out = pool.tile([128, KD, CH], BF, tag="t4" if pool is t4 else "xft")
            for d in range(KD):
                for tt in range(2):
                    # bank-sized bf16 tile so it shares the "mm" tag key
                    ps = mmp.tile([128, 1024], BF, tag="mm")
                    nc.tensor.transpose(ps[:, 0:128], src[:, tt, ds(d * 128, 128)],
                                        ident[:])
                    nc.vector.tensor_copy(out[:, d, ds(tt * 128, 128)],
                                          ps[:, 0:128])
            return out

        def load_wqkvo(l):
            w = wqkvop.tile([128, KD, 4, 512], BF, tag="wqkvo", name=f"wqkvo{l}")
            for i, src in enumerate((wq, wk, wv, wo)):
                nc.sync.dma_start(
                    w[:, :, i, :],
                    src[l].rearrange("(ko p) m -> p ko m", p=128))
            return w

        def load_w1(l):
            w = w1pool.tile([128, KD, FF], BF, tag="w1k", name=f"w1k{l}")
            nc.sync.dma_start(
                w[:], w1[l].rearrange("(ko p) f -> p ko f", p=128))
            return w

        # ---- preload layer 0 weights, then the resident LM head weights ----
        cur_wqkvo = load_wqkvo(0)
        cur_w1 = load_w1(0)
        wlm_sb = wlmp.tile([128, KD, VS], BF)
        nc.sync.dma_start(wlm_sb[:], wlm.rearrange("(ko p) v -> p ko v", p=128))

        xt = xpool.tile([128, 2, D], FP, tag="xt")
        nc.sync.dma_start(xt[:], x0.rearrange("(tt p) d -> p tt d", p=128))

        nxt_wqkvo = nxt_w1 = None
        for l in range(layers):
            # ---- LN1 + transpose ----
            h = layernorm(xt, "h")
            hT = transpose2(h)

            # ---- k/v first so the AllGather starts ASAP ----
            kT = t4.tile([128, KD, CH], BF, tag="t4")
            for d in range(KD):
                ps = mmp.tile([128, 512], FP, tag="mm")
                for k in range(KD):
                    nc.tensor.matmul(ps[:, :CH], cur_wqkvo[:, k, 1, ds(d * 128, 128)],
                                     hT[:, k, :], start=(k == 0),
                                     stop=(k == KD - 1))
                if with_bias:
                    nc.vector.tensor_scalar_add(kT[:, d, :], ps[:, :CH],
                                                bqk_sb[:, l, 1, d:d + 1])
                else:
                    nc.vector.tensor_copy(kT[:, d, :], ps[:, :CH])
            vaug = vaugp.tile([128, 2, H, 65], BF)
            nc.vector.memset(vaug[:, :, :, 64:65], 1.0)
            for tt in range(2):
                ps = mmp.tile([128, 512], FP, tag="mm")
                for k in range(KD):
                    nc.tensor.matmul(ps[:, :D], hT[:, k, ds(tt * 128, 128)],
                                     cur_wqkvo[:, k, 2, :], start=(k == 0),
                                     stop=(k == KD - 1))
                nc.vector.tensor_copy(
                    vaug[:, tt, :, 0:64],
                    ps[:].rearrange("p (h e) -> p h e", h=H))

            # ---- AllGather K,V across batch group (bf16) ----
            kv_in = dram.tile([KV], BF, tag="kvin")
            nc.sync.dma_start(
                kv_in[0:KV_K].rearrange("(p a b) -> p a b", p=128, a=KD), kT[:])
            nc.sync.dma_start(
                kv_in[KV_K:KV].rearrange("(p a h e) -> p a h e", p=128, a=2, h=H),
                vaug[:])
            kv_out = dram.tile([4, KV], BF, tag="kvout")
            nc.gpsimd.collective_compute(
                "AllGather", OP.bypass,
                replica_groups=[[0, 1, 2, 3], [4, 5, 6, 7]],
                ins=[kv_in[:].opt()], outs=[kv_out[:].opt()])

            # ---- q projection + weight prefetch overlap the collective ----
            qT = t4.tile([128, KD, CH], BF, tag="t4")
            for d in range(KD):
                ps = mmp.tile([128, 512], FP, tag="mm")
                for k in range(KD):
                    nc.tensor.matmul(ps[:, :CH], cur_wqkvo[:, k, 0, ds(d * 128, 128)],
                                     hT[:, k, :], start=(k == 0),
                                     stop=(k == KD - 1))
                if with_bias:
                    nc.vector.tensor_scalar_add(qT[:, d, :], ps[:, :CH],
                                                bqk_sb[:, l, 0, d:d + 1])
                else:
                    nc.vector.tensor_copy(qT[:, d, :], ps[:, :CH])
            if l + 1 < layers:
                nxt_wqkvo = load_wqkvo(l + 1)
                nxt_w1 = load_w1(l + 1)

            kTall = kvall.tile([128, KD, 4, CH], BF, tag="ktall")
            vall = kvall.tile([128, 4, 2, H, 65], BF, tag="vall")
            for c in range(4):
                nc.sync.dma_start(
                    kTall[:, :, c, :],
                    kv_out[c, 0:KV_K].rearrange("(p a b) -> p a b", p=128, a=KD))
                nc.sync.dma_start(
                    vall[:, c, :, :, :],
                    kv_out[c, KV_K:KV].rearrange("(p a h e) -> p a h e",
                                                 p=128, a=2, h=H))

            # ---- attention: per-head PSUM accumulation over the 4 chunks ----
            # att rows 0-63 hold numerators; row 64 holds the denominators
            att = attp.tile([128, H, CH], BF, tag="attn", name=f"att{l}")
            for hh in range(H):
                pb = (hh % 2) * 64
                dt_ = hh // 2
                avps = avp.tile([65, CH], FP, tag="av")
                for c in range(4):
                    sps = spp.tile([128, 2, CH], FP, tag="sp")
                    for kt in range(2):
                        nc.tensor.matmul(
                            sps[:, kt, :],
                            kTall[pb:pb + 64, dt_, c, ds(kt * 128, 128)],
                            qT[pb:pb + 64, dt_, :],
                            start=True, stop=True)
                    ex = expp.tile([128, 2, CH], BF, tag="exp")
                    nc.scalar.activation(ex[:], sps[:], AF.Exp)
                    nc.gpsimd.tensor_tensor(ex[:], ex[:], msk_sb[:, c, :, :],
                                            OP.mult)
                    for kt in range(2):
                        nc.tensor.matmul(
                            avps[:], vall[:, c, kt, hh, :], ex[:, kt, :],
                            start=(c == 0 and kt == 0),
                            stop=(c == 3 and kt == 1))
                nc.vector.tensor_copy(att[0:64, hh, :], avps[0:64, :])
                nc.vector.tensor_copy(att[64:65, hh, :], avps[64:65, :])

            # ---- normalize: compact reciprocal then DMA-broadcast ----
            rdram = dram.tile([H * CH], BF, tag="rdram")
            nc.sync.dma_start(
                rdram[:].rearrange("(a h c) -> a h c", a=1, h=H),
                att[64:65, :, :])
            rsb = small.tile([128, H * CH // 128], BF, tag="rsb")
            nc.sync.dma_start(rsb[:], rdram[:].rearrange("(p f) -> p f", p=128))
            rb16 = small.tile([128, H * CH // 128], BF, tag="rb16")
            with nc.allow_low_precision(reason="softmax denom reciprocal in bf16"):
                nc.vector.reciprocal(rb16[:], rsb[:])
            rdram2 = dram.tile([H * CH], BF, tag="rdram2")
            nc.sync.dma_start(rdram2[:].rearrange("(p f) -> p f", p=128), rb16[:])
            rbc = attp.tile([64, H, CH], BF, tag="rbc", name=f"rbc{l}")
            for hh in range(H):
                nc.sync.dma_start(
                    rbc[:, hh, :],
                    bass.AP(tensor=rdram2.tensor, offset=rdram2.offset + hh * CH,
                            ap=[[0, 64], [1, CH]]))
            nc.vector.tensor_tensor(att[0:64, :, :], att[0:64, :, :], rbc[:],
                                    OP.mult)
            attT = t4.tile([128, KD, CH], BF, tag="t4")
            for dt_ in range(KD):
                for a in range(2):
                    nc.sync.dma_start(attT[a * 64:(a + 1) * 64, dt_, :],
                                      att[0:64, 2 * dt_ + a, :])

            # ---- Wo + bias + residual ----
            if with_bias:
                bo_b = bcp.tile([128, D], FP, tag="bc")
                bo_src = bo2[l, 0]
                nc.sync.dma_start(bo_b[:], bass.AP(
                    tensor=bo_src.tensor, offset=bo_src.offset,
                    ap=[[0, 128]] + list(bo_src.ap)))
            for tt in range(2):
                ps = mmp.tile([128, 512], FP, tag="mm")
                for k in range(KD):
                    nc.tensor.matmul(ps[:, :D], attT[:, k, ds(tt * 128, 128)],
                                     cur_wqkvo[:, k, 3, :], start=(k == 0),
                                     stop=(k == KD - 1))
                if with_bias:
                    nc.vector.tensor_tensor(ps[:, :D], ps[:, :D], bo_b[:],
                                            OP.add)
                nc.vector.tensor_tensor(xt[:, tt, :], xt[:, tt, :], ps[:, :D],
                                        OP.add)

            # ---- LN2 + transpose ----
            h2 = layernorm(xt, "h")
            h2T = transpose2(h2)

            # ---- MLP (gelu tiles packed 2 m-tiles per activation) ----
            if with_bias:
                b2_b = bcp.tile([128, D], FP, tag="bc")
                b2_src = bo2[l, 1]
                nc.sync.dma_start(b2_b[:], bass.AP(
                    tensor=b2_src.tensor, offset=b2_src.offset,
                    ap=[[0, 128]] + list(b2_src.ap)))
            x2ps = [mop.tile([128, D], FP, tag="mo", name=f"mo{l}_{kk}")
                    for kk in range(2)]
            for mp in range(MD // 2):
                gps = mmp.tile([128, 2, CH], FP, tag="mm")
                for mi in range(2):
                    m = mp * 2 + mi
                    for k in range(KD):
                        nc.tensor.matmul(gps[:, mi, :],
                                         cur_w1[:, k, ds(m * 128, 128)],
                                         h2T[:, k, :], start=(k == 0),
                                         stop=(k == KD - 1))
                gt = gtp.tile([128, 2, CH], BF, tag="gt")
                if with_bias:
                    for mi in range(2):
                        m = mp * 2 + mi
                        nc.scalar.activation(gt[:, mi, :], gps[:, mi, :], AF.Gelu,
                                             bias=b1_sb[:, l, m:m + 1])
                else:
                    nc.scalar.activation(gt[:], gps[:], AF.Gelu)
                for mi in range(2):
                    m = mp * 2 + mi
                    w2t = w2pool.tile([128, D], BF, tag="w2k")
                    nc.sync.dma_start(
                        w2t[:], w2[l].rearrange("(ko p) d -> p ko d", p=128)[:, m, :])
                    for tt in range(2):
                        nc.tensor.matmul(x2ps[tt][:], gt[:, mi, ds(tt * 128, 128)],
                                         w2t[:], start=(m == 0), stop=(m == MD - 1))
            for tt in range(2):
                if with_bias:
                    nc.vector.tensor_tensor(x2ps[tt][:], x2ps[tt][:], b2_b[:],
                                            OP.add)
                nc.vector.tensor_tensor(xt[:, tt, :], xt[:, tt, :], x2ps[tt][:],
                                        OP.add)
            cur_wqkvo, cur_w1 = nxt_wqkvo, nxt_w1

        # ---- final LN; AllGather hidden state across all 8 (Shared/RDH) ----
        xf = layernorm(xt, "h")
        xfT = transpose2(xf, pool=xftp)
        xf_in = dram.tile([XF], BF, tag="xfin")
        nc.sync.dma_start(
            xf_in[:].rearrange("(p a b) -> p a b", p=128, a=KD), xfT[:])
        xf_out = dram.tile([NC, XF], BF, tag="xfout", addr_space="Shared")
        nc.gpsimd.collective_compute(
            "AllGather", OP.bypass,
            replica_groups=[list(range(NC))],
            ins=[xf_in[:].opt()], outs=[xf_out[:].opt()])

        NT = (VS + 511) // 512
        hp_pools = ((mmp, "mm"), (mop, "mo"), (spp, "sp"))
        gi = 0

        def head_block(xa, cslot, n, nsz, rowbase):
            """rowbase None => warmup-only: compute, discard (keeps PE busy)."""
            nonlocal gi
            for mt in range(2):
                pool_i, ptag = hp_pools[gi % 3]
                ps = pool_i.tile([128, 512], FP, tag=ptag,
                                 name=f"hps{n}_{rowbase}_{mt}")
                for k in range(KD):
                    nc.tensor.matmul(
                        ps[:, :nsz],
                        xa[:, k, cslot, ds(mt * 128, 128)] if cslot is not None
                        else xa[:, k, ds(mt * 128, 128)],
                        wlm_sb[:, k, ds(n * 512, nsz)],
                        start=(k == 0), stop=(k == KD - 1))
                if rowbase is None:
                    gi += 1
                    continue
                lg = lgp.tile([128, 512], BF, tag="lg",
                              name=f"lg{n}_{rowbase}_{mt}")
                if gi % 2 == 0:
                    nc.vector.tensor_copy(lg[:, :nsz], ps[:, :nsz])
                else:
                    nc.scalar.activation(lg[:, :nsz], ps[:, :nsz], AF.Copy)
                gi += 1
                nc.sync.dma_start(
                    logits[ds(rowbase + mt * 128, 128), ds(n * 512, nsz)],
                    lg[:, :nsz])

        # pass 1: warmup on the local tokens while the AllGather flies;
        # results are discarded (pass 2 recomputes every chunk uniformly)
        for n in range(NT):
            head_block(xfT, None, n, min(512, VS - n * 512), None)

        xfall = []
        for cg in range(2):
            xa = xfap.tile([128, KD, 4, CH], BF, tag="xfa", name=f"xfa{cg}")
            for c in range(4):
                nc.sync.dma_start(
                    xa[:, :, c, :],
                    xf_out[cg * 4 + c, :].rearrange("(p a b) -> p a b",
                                                    p=128, a=KD))
            xfall.append(xa)
        for n in range(NT):
            nsz = min(512, VS - n * 512)
            for c in range(NC):
                head_block(xfall[c // 4], c % 4, n, nsz, c * CH)

    nc.compile()
    return nc


_CACHE = {}


def _get_program(with_bias=True):
    key = ("nc", with_bias)
    if key not in _CACHE:
        _CACHE[key] = build_program(with_bias=with_bias)
    return _CACHE[key]


def _prep_inputs(inputs):
    f = lambda k: np.asarray(inputs[k], np.float32)
    bf = ml_dtypes.bfloat16
    idx = np.asarray(inputs["idx"]).astype(np.int64)
    tok_emb, pos_emb = f("tok_emb"), f("pos_emb")
    x0 = tok_emb[idx] + pos_emb[None, :T]          # [B, T, D]
    x0 = x0.reshape(NC, CH, D)

    ln1_g, ln1_b = f("ln1_g"), f("ln1_b")
    ln2_g, ln2_b = f("ln2_g"), f("ln2_b")
    Wq, bq = f("Wq"), f("bq")
    Wk, bk = f("Wk"), f("bk")
    Wv, bv = f("Wv"), f("bv")
    Wo, bo = f("Wo"), f("bo")
    W1, b1 = f("W1"), f("b1")
    W2, b2 = f("W2"), f("b2")
    lnf_g, lnf_b = f("lnf_g"), f("lnf_b")
    Wlm, blm = f("Wlm"), f("blm")

    sc = 1.0 / np.sqrt(HS)
    wqe = ln1_g[:, :, None] * Wq * sc
    bqe = (np.einsum("ld,ldm->lm", ln1_b, Wq) + bq) * sc
    wke = ln1_g[:, :, None] * Wk
    bke = np.einsum("ld,ldm->lm", ln1_b, Wk) + bk
    wve = ln1_g[:, :, None] * Wv
    bve = np.einsum("ld,ldm->lm", ln1_b, Wv) + bv
    boe = np.einsum("lm,lmd->ld", bve, Wo) + bo
    w1e = ln2_g[:, :, None] * W1
    b1e = np.einsum("ld,ldf->lf", ln2_b, W1) + b1
    wlme = lnf_g[:, None] * Wlm
    blme = lnf_b @ Wlm + blm

    bqk = np.stack([bqe, bke], axis=1)             # [L, 2, D]
    bqk = bqk.reshape(L, 2, KD, 128).transpose(3, 0, 1, 2).copy()
    b1t = b1e.reshape(L, MD, 128).transpose(2, 0, 1).copy()
    bo2 = np.stack([boe, b2], axis=1)              # [L, 2, D]

    wlmp = np.zeros((D, NC * VS), np.float32)
    wlmp[:, :V] = wlme

    # causal 0/1 masks per core (multiplied in after exp): [p, kc, kt, q]
    masks = []
    for core in range(NC):
        cc = core % 4
        qpos = cc * CH + np.arange(CH)
        m = np.empty((128, 4, 2, CH), np.float32)
        for kc in range(4):
            for kt in range(2):
                kpos = kc * CH + kt * 128 + np.arange(128)
                m[:, kc, kt, :] = (kpos[:, None] <= qpos[None, :]).astype(np.float32)
        masks.append(m.astype(bf))

    shared = dict(wq=np.ascontiguousarray(wqe.astype(bf)),
                  wk=np.ascontiguousarray(wke.astype(bf)),
                  wv=np.ascontiguousarray(wve.astype(bf)),
                  wo=np.ascontiguousarray(Wo.astype(bf)),
                  w1=np.ascontiguousarray(w1e.astype(bf)),
                  w2=np.ascontiguousarray(W2.astype(bf)),
                  bqk=bqk, b1t=b1t, bo2=np.ascontiguousarray(bo2))
    in_maps = []
    for core in range(NC):
        m = dict(shared)
        m["x0"] = np.ascontiguousarray(x0[core])
        m["msk"] = masks[core]
        m["wlm"] = np.ascontiguousarray(
            wlmp[:, core * VS:(core + 1) * VS].astype(bf))
        in_maps.append(m)
    return in_maps, blme


def _run(inputs, trace=False):
    in_maps, blme = _prep_inputs(inputs)
    with_bias = bool(np.any(in_maps[0]["bo2"]))
    nc = _get_program(with_bias=with_bias)
    res = bass_utils.run_bass_kernel_spmd(nc, in_maps, core_ids=list(range(NC)),
                                          trace=trace)
    lg = np.concatenate(
        [np.asarray(res.results[c]["logits"]).astype(np.float32)
         for c in range(NC)], axis=1)
    out = lg[:, :V]
    if np.any(blme):
        out = out + blme[None, :]
    return out.reshape(B, T, V).astype(np.float32), res


def kernel(**inputs) -> np.ndarray:
    out, _ = _run(inputs, trace=False)
    return out
